# revision 23
# baseline (speedup 1.0000x reference)
"""Trainium2 Bass kernel for the DynamicBlock (ragged top-k decoder layer).

Sharding: 8 cores = (batch b in 0..3) x (query-half h in 0..1).
Core (b, h) processes queries k in [h*512, (h+1)*512) of the K=1024 selected
rows of batch b (causal: needs K/V for all 1024 selected rows, computed
locally -- no collectives).  Matmuls run in bf16 with fp32 accumulation;
norms/softmax/residual/gating in fp32.

Key structure (v2):
- keys permuted own-half-first; the other half is either fully allowed
  (h=1) or fully masked (h=0), folded into the EXP bias per core; only the
  4 own diagonal blocks need an explicit triangular mask multiply.
- scores/exp/attn matmuls trimmed to the causal column range on own tiles.
- softmax 1/sum via reciprocal_approx_fast (single DVE op) straight out of
  PSUM; pso/pss live in a dedicated 2-bank PSUM pool so the next head's
  score matmuls never WAR-wait on the normalization chain.
- B/C interleaved for in-order engines: own gathers -> Q proj -> other
  gathers -> V -> K, so the PE starts ~40us earlier.
- weight pools for o-proj/down-proj created one phase early so their first
  DMAs prefetch across the phase boundary.
- untouched hidden rows are assembled host-side (no device copy-through).
"""

import math
from contextlib import ExitStack
from dataclasses import dataclass

import ml_dtypes
import numpy as np

import concourse.bass as bass
import concourse.mybir as mybir
import concourse.tile as tile
from concourse import bacc
from concourse.bass import IndirectOffsetOnAxis

P = 128
F32 = mybir.dt.float32
BF16 = mybir.dt.bfloat16
I32 = mybir.dt.int32
AF = mybir.ActivationFunctionType
BF16NP = ml_dtypes.bfloat16


@dataclass(frozen=True)
class Cfg:
    T: int = 4096      # full sequence length
    D: int = 2048      # model dim
    KSEL: int = 1024   # selected rows per sequence
    H: int = 16        # query heads
    KVH: int = 4       # kv heads
    HD: int = 128      # head dim (must equal P)
    FF: int = 8192     # mlp intermediate
    EPS: float = 1e-6

    @property
    def DT(self):
        return self.D // P

    @property
    def QROWS(self):
        return self.KSEL // 2

    @property
    def QT(self):
        return self.QROWS // P

    @property
    def KT(self):
        return self.KSEL // P

    @property
    def FFT(self):
        return self.FF // P

    @property
    def FFG(self):
        return self.FFT // 4

    @property
    def T2(self):
        return self.T // 2

    @property
    def GQ(self):
        return self.H // self.KVH


FULL = Cfg()


def _chunks(total, size):
    out = []
    s = 0
    while s < total:
        out.append((s, min(size, total - s)))
        s += size
    return out


def emit(nc: bass.Bass, c: Cfg, upto: str = "G"):
    DT, QT, KT, QROWS, KVD = c.DT, c.QT, c.KT, c.QROWS, c.KVH * c.HD
    OGS = _chunks(c.D, 512)  # output-column groups for o-proj / down-proj

    # ---- DRAM I/O ----
    hid_d = nc.dram_tensor("hid", [c.T, c.D], F32, kind="ExternalInput")
    idxkv_d = nc.dram_tensor("idx_kv", [P, KT], I32, kind="ExternalInput")
    gsc_d = nc.dram_tensor("gsc", [P, QT], F32, kind="ExternalInput")
    cos_d = nc.dram_tensor("cosb", [c.T, c.HD], F32, kind="ExternalInput")
    sin_d = nc.dram_tensor("sinb", [c.T, c.HD], F32, kind="ExternalInput")
    tri_d = nc.dram_tensor("trim", [P, P], BF16, kind="ExternalInput")
    biasm_d = nc.dram_tensor("biasm", [P, 1], F32, kind="ExternalInput")
    wq_d = nc.dram_tensor("wq", [c.H, P, DT, c.HD], BF16, kind="ExternalInput")
    wk_d = nc.dram_tensor("wk", [c.KVH, P, DT, c.HD], BF16, kind="ExternalInput")
    wv_d = nc.dram_tensor("wv", [P, DT, KVD], BF16, kind="ExternalInput")
    wo_d = nc.dram_tensor("wo", [len(OGS), c.H, P, OGS[0][1]], BF16, kind="ExternalInput")
    wg_d = nc.dram_tensor("wg", [c.FFG, DT, P, 512], BF16, kind="ExternalInput")
    wu_d = nc.dram_tensor("wu", [c.FFG, DT, P, 512], BF16, kind="ExternalInput")
    wd_d = nc.dram_tensor("wd", [len(OGS), c.FFT, P, OGS[0][1]], BF16, kind="ExternalInput")
    bq_d = nc.dram_tensor("bq", [P, c.H], F32, kind="ExternalInput")
    bk_d = nc.dram_tensor("bk", [P, c.KVH], F32, kind="ExternalInput")
    bv_d = nc.dram_tensor("bv", [1, KVD], F32, kind="ExternalInput")
    idf_d = nc.dram_tensor("id_f", [P, P], F32, kind="ExternalInput")
    idb_d = nc.dram_tensor("id_b", [P, P], BF16, kind="ExternalInput")
    perm_d = nc.dram_tensor("perm", [P, P], BF16, kind="ExternalInput")
    ones_d = nc.dram_tensor("ones_b", [P, P], BF16, kind="ExternalInput")

    oupd_d = nc.dram_tensor("out_upd", [QROWS, c.D], F32, kind="ExternalOutput")

    scl = 1.0 / math.sqrt(c.HD)

    with ExitStack() as top:
        tc = top.enter_context(tile.TileContext(nc))
        constp = top.enter_context(tc.tile_pool(name="constp", bufs=1, side="left"))
        residp = top.enter_context(tc.tile_pool(name="residp", bufs=1, side="left"))
        psp = top.enter_context(tc.tile_pool(name="psp", bufs=6, space="PSUM"))

        def ps_tile():
            return psp.tile([P, 512], F32, tag="ps", name="ps")

        # ---- constants (indices first: they gate the gathers) ----
        idxkv = constp.tile([P, KT], I32, tag="idxkv")
        nc.sync.dma_start(idxkv[:], idxkv_d[:])
        idf = constp.tile([P, P], F32, tag="idf")
        nc.sync.dma_start(idf[:], idf_d[:])
        idb = constp.tile([P, P], BF16, tag="idb")
        nc.sync.dma_start(idb[:], idb_d[:])
        perm = constp.tile([P, P], BF16, tag="perm")
        nc.sync.dma_start(perm[:], perm_d[:])
        ones_b = constp.tile([P, P], BF16, tag="ones_b")
        nc.sync.dma_start(ones_b[:], ones_d[:])
        tri = constp.tile([P, P], BF16, tag="tri")
        nc.sync.dma_start(tri[:], tri_d[:])
        biasm = constp.tile([P, 1], F32, tag="biasm")
        nc.sync.dma_start(biasm[:], biasm_d[:])
        gsc = constp.tile([P, QT], F32, tag="gsc")
        nc.sync.dma_start(gsc[:], gsc_d[:])
        bqc = constp.tile([P, c.H], F32, tag="bqc")
        nc.sync.dma_start(bqc[:], bq_d[:])
        bkc = constp.tile([P, c.KVH], F32, tag="bkc")
        nc.sync.dma_start(bkc[:], bk_d[:])
        epsc = constp.tile([P, 1], F32, tag="epsc")
        nc.vector.memset(epsc[:], c.EPS)
        bvbc = constp.tile([P, KVD], F32, tag="bvbc")
        bv_ap = bv_d[:]
        nc.sync.dma_start(
            bvbc[:], bass.AP(tensor=bv_ap.tensor, offset=0, ap=[[0, P], [1, KVD]])
        )

        # residual (live until the end)
        xq_raw = residp.tile([P, QT, c.D], F32, tag="xq_raw")

        sgw = math.gcd(512, c.D)
        nsub = c.D // sgw

        es_bt = ExitStack()  # xkvT/xqT/cos/sin: freed after projections
        xtp = es_bt.enter_context(tc.tile_pool(name="xtp", bufs=1, side="left"))
        xkvT = xtp.tile([P, DT, c.KSEL], BF16, tag="xkvT")
        cosTkv = xtp.tile([P, c.KSEL], F32, tag="cosTkv")
        sinTkv = xtp.tile([P, c.KSEL], F32, tag="sinTkv")
        cosg = xtp.tile([P, KT, c.HD], F32, tag="cosg")
        sing = xtp.tile([P, KT, c.HD], F32, tag="sing")
        # host permutes the key order so this core's own query half is rows
        # [0, QROWS) -- q-side tensors are static slices of the kv tensors
        xqT = xkvT[:, :, :QROWS]
        cosTq = cosTkv[:, :QROWS]
        sinTq = sinTkv[:, :QROWS]

        es_qkv = ExitStack()
        qkvp = es_qkv.enter_context(tc.tile_pool(name="qkvp", bufs=1, side="right"))
        kT = qkvp.tile([P, c.KVH, c.KSEL], BF16, tag="kT")
        vN = qkvp.tile([P, KT, KVD], BF16, tag="vN")
        qT = qkvp.tile([P, c.H, QROWS], BF16, tag="qT")

        def gather_rows(dst, src_dram, col, split=1):
            """Indirect row gather; dst slots are always fresh so the only
            dependency is the idx tile (single sync wait on the dyn queue).
            split>1 breaks the row into column chunks so more packets are in
            flight per DMA engine (hides per-descriptor latency)."""
            ncols = src_dram.shape[-1]
            step = ncols // split
            for s in range(split):
                nc.gpsimd.indirect_dma_start(
                    out=dst[:, s * step : (s + 1) * step],
                    out_offset=None,
                    in_=src_dram[:],
                    in_offset=IndirectOffsetOnAxis(ap=idxkv[:, col : col + 1], axis=0),
                    element_offset=s * step,
                )

        with tc.tile_pool(name="gpool", bufs=3, side="left") as gpool, \
             tc.tile_pool(name="spool", bufs=4, side="left") as spool, \
             tc.tile_pool(name="psbp", bufs=2, space="PSUM") as psbp, \
             tc.tile_pool(name="wstr", bufs=2, side="left") as wstr, \
             tc.tile_pool(name="rpool", bufs=2, side="left") as rpool:

            def psb_tile():
                return psbp.tile([P, P], BF16, tag="psb", name="psb")

            def norm_transpose(raw, xn_out_fn):
                """raw: [P, D] f32 tile; writes bf16 normalized transposed tiles."""
                stats = spool.tile([P, nsub, 6], F32, tag="stats")
                for s in range(nsub):
                    nc.vector.bn_stats(stats[:, s, :], raw[:, s * sgw : (s + 1) * sgw])
                mv = spool.tile([P, 2], F32, tag="mv")
                nc.vector.bn_aggr(mv[:], stats[:])
                msq = spool.tile([P, 1], F32, tag="msq")
                nc.vector.tensor_mul(msq[:], mv[:, 0:1], mv[:, 0:1])
                nc.vector.tensor_add(msq[:], msq[:], mv[:, 1:2])
                srt = spool.tile([P, 1], F32, tag="srt")
                nc.scalar.activation(srt[:], msq[:], AF.Sqrt, bias=epsc[:])
                rstd = spool.tile([P, 1], F32, tag="rstd")
                nc.vector.reciprocal(rstd[:], srt[:])
                xn = gpool.tile([P, c.D], BF16, tag="xn")
                nc.vector.tensor_scalar_mul(xn[:], raw[:], rstd[:])
                for dt in range(DT):
                    tp = psb_tile()
                    nc.tensor.transpose(tp[:], xn[:, dt * P : (dt + 1) * P], idb[:])
                    # alternate copy engine so neither serializes the drain
                    if dt % 2 == 0:
                        nc.scalar.copy(xn_out_fn(dt), tp[:])
                    else:
                        nc.vector.tensor_copy(xn_out_fn(dt), tp[:])

            def cs_transpose(t):
                for ei, (src, dst) in enumerate(((cosg, cosTkv), (sing, sinTkv))):
                    tp = ps_tile()
                    nc.tensor.transpose(tp[:, :P], src[:, t, :], idf[:])
                    if ei == 0:
                        nc.scalar.copy(dst[:, t * P : (t + 1) * P], tp[:, :P])
                    else:
                        nc.vector.tensor_copy(dst[:, t * P : (t + 1) * P], tp[:, :P])

            def rope(dst, rawt, rot_ps, cosT, sinT, s0, w):
                t1 = rpool.tile([P, 512], F32, tag="ropet1")
                nc.vector.tensor_mul(t1[:, :w], rawt[:, s0 : s0 + w], cosT[:, s0 : s0 + w])
                t2 = rpool.tile([P, 512], F32, tag="ropet2")
                nc.vector.tensor_mul(t2[:, :w], rot_ps[:, :w], sinT[:, s0 : s0 + w])
                nc.vector.tensor_add(dst[:, s0 : s0 + w], t1[:, :w], t2[:, :w])

            # ---- phase B1: own-half gathers + rmsnorm1 + transpose ----
            with nc.named_scope("B1"):
                for t in range(QT):
                    raw = xq_raw[:, t, :]
                    gather_rows(raw, hid_d, t, split=2)
                    norm_transpose(
                        raw, lambda dt, t=t: xkvT[:, dt, t * P : (t + 1) * P]
                    )
                for t in range(QT):
                    gather_rows(cosg[:, t, :], cos_d, t)
                    gather_rows(sing[:, t, :], sin_d, t)

            # ---- phase C1: Q projection + rope ----
            with nc.named_scope("C1"):
                for m in range(c.H):
                    wqm = wstr.tile([P, DT, c.HD], BF16, tag="wqkm", bufs=3)
                    nc.sync.dma_start(wqm[:], wq_d[m])
                    qraw = rpool.tile([P, QROWS], BF16, tag="kqraw", name="qraw")
                    ps = ps_tile()
                    for dt in range(DT):
                        nc.tensor.matmul(
                            ps[:, :QROWS],
                            wqm[:, dt, :],
                            xqT[:, dt, :],
                            start=(dt == 0),
                            stop=(dt == DT - 1),
                        )
                    nc.vector.tensor_scalar_add(
                        qraw[:], ps[:, :QROWS], bqc[:, m : m + 1]
                    )
                    if m == 0:
                        # own cos/sin transposes: data has landed by the time
                        # the PE finishes head 0's projection
                        for t in range(QT):
                            cs_transpose(t)
                    rot = ps_tile()
                    nc.tensor.matmul(
                        rot[:, :QROWS], perm[:], qraw[:], start=True, stop=True
                    )
                    rope(qT[:, m, :], qraw, rot, cosTq, sinTq, 0, QROWS)

            # ---- phase B2 + C2a: other-half gathers interleaved with V ----
            # V-proj matmuls for the already-transposed B1 tiles run while
            # the other-half gathers land; each B2 tile's norm+transposes
            # then interleave with the next V-proj block so the PE never
            # sits in a copy-paced transpose run
            with nc.named_scope("B2"):
                wvsb = wstr.tile([P, DT, KVD], BF16, tag="wvsb", bufs=1)
                nc.sync.dma_start(wvsb[:], wv_d[:])
                graws = {}
                for t in range(QT, KT):
                    graws[t] = gpool.tile(
                        [P, c.D], F32, tag="graw", name="graw", bufs=3
                    )[:]
                    gather_rows(graws[t], hid_d, t, split=2)
                for t in range(QT, KT):
                    gather_rows(cosg[:, t, :], cos_d, t)
                    gather_rows(sing[:, t, :], sin_d, t)

                def v_proj(rt):
                    psv = ps_tile()
                    for dt in range(DT):
                        nc.tensor.matmul(
                            psv[:, :KVD],
                            xkvT[:, dt, rt * P : (rt + 1) * P],
                            wvsb[:, dt, :],
                            start=(dt == 0),
                            stop=(dt == DT - 1),
                        )
                    nc.vector.tensor_add(vN[:, rt, :], psv[:, :KVD], bvbc[:])

                for rt in range(QT):
                    v_proj(rt)
                for t in range(QT, KT):
                    norm_transpose(
                        graws[t], lambda dt, t=t: xkvT[:, dt, t * P : (t + 1) * P]
                    )
                    v_proj(t)

            # ---- phase C2: K projection + rope ----
            with nc.named_scope("C2"):
                for t in range(QT, KT):
                    cs_transpose(t)
                for m in range(c.KVH):
                    wkm = wstr.tile([P, DT, c.HD], BF16, tag="wqkm", bufs=3)
                    nc.sync.dma_start(wkm[:], wk_d[m])
                    kraw = rpool.tile([P, c.KSEL], BF16, tag="kraw")
                    for s0, w in _chunks(c.KSEL, 512):
                        ps = ps_tile()
                        for dt in range(DT):
                            nc.tensor.matmul(
                                ps[:, :w],
                                wkm[:, dt, :],
                                xkvT[:, dt, s0 : s0 + w],
                                start=(dt == 0),
                                stop=(dt == DT - 1),
                            )
                        nc.vector.tensor_scalar_add(
                            kraw[:, s0 : s0 + w], ps[:, :w], bkc[:, m : m + 1]
                        )
                    for s0, w in _chunks(c.KSEL, 512):
                        rot = ps_tile()
                        nc.tensor.matmul(
                            rot[:, :w], perm[:], kraw[:, s0 : s0 + w], start=True, stop=True
                        )
                        rope(kT[:, m, :], kraw, rot, cosTkv, sinTkv, s0, w)

        es_bt.close()  # free xkvT/xqT/cos/sin

        # ---- phase D: attention ----
        es_res2 = ExitStack()  # attn+mlp residual, lives D -> G
        res2p = es_res2.enter_context(tc.tile_pool(name="res2p", bufs=1, side="left"))
        res2 = res2p.tile([P, QT, c.D], F32, tag="res2")
        es_attn = ExitStack()
        attnp = es_attn.enter_context(tc.tile_pool(name="attnp", bufs=1, side="left"))
        xattnT = attnp.tile([P, c.H, QROWS], BF16, tag="xattnT")

        # o-proj weight pool created BEFORE D: its first DMAs prefetch
        # during the (DMA-idle) attention phase
        es_wo = ExitStack()
        wstr2 = es_wo.enter_context(tc.tile_pool(name="wstr2", bufs=16, side="left"))
        NPRE_O = 16
        wot_pre = []
        for ht in range(NPRE_O):
            wot = wstr2.tile([P, OGS[0][1]], BF16, tag="wot", name="wot")
            nc.sync.dma_start(wot[:], wo_d[0, ht])
            wot_pre.append(wot)

        with nc.named_scope("D"), \
             tc.tile_pool(name="dpool", bufs=3, side="left") as dpool, \
             tc.tile_pool(name="psov", bufs=2, space="PSUM") as psov, \
             tc.tile_pool(name="rcpool", bufs=2, side="left") as rcpool:
            for h in range(c.H):
                g = h // c.GQ
                expT = dpool.tile([P, KT, QROWS], BF16, tag="expT")
                # own-half tiles: causal-trimmed columns + diagonal tri mask
                for j in range(QT):
                    s0 = j * P
                    ps = ps_tile()
                    nc.tensor.matmul(
                        ps[:, s0:QROWS],
                        kT[:, g, s0 : s0 + P],
                        qT[:, h, s0:QROWS],
                        start=True,
                        stop=True,
                    )
                    nc.scalar.activation(
                        expT[:, j, s0:QROWS], ps[:, s0:QROWS], AF.Exp, scale=scl
                    )
                    nc.vector.tensor_mul(
                        expT[:, j, s0 : s0 + P], expT[:, j, s0 : s0 + P], tri[:]
                    )
                # other-half tiles: all-allowed (h=1) or all-masked (h=0),
                # folded into the exp bias (e^-60 ~ 0)
                for j in range(QT, KT):
                    ps = ps_tile()
                    nc.tensor.matmul(
                        ps[:, :QROWS],
                        kT[:, g, j * P : (j + 1) * P],
                        qT[:, h, :],
                        start=True,
                        stop=True,
                    )
                    nc.scalar.activation(
                        expT[:, j, :], ps[:, :QROWS], AF.Exp,
                        scale=scl, bias=biasm[:],
                    )
                pso = psov.tile([P, 512], F32, tag="pso", name="pso")
                pss = psov.tile([P, 512], F32, tag="pso", name="pss")
                for j in range(KT):
                    s0 = j * P if j < QT else 0
                    nc.tensor.matmul(
                        pso[:, s0:QROWS],
                        vN[:, j, g * c.HD : (g + 1) * c.HD],
                        expT[:, j, s0:QROWS],
                        start=(j == 0),
                        stop=(j == KT - 1),
                    )
                for j in range(KT):
                    s0 = j * P if j < QT else 0
                    nc.tensor.matmul(
                        pss[:, s0:QROWS],
                        ones_b[:],
                        expT[:, j, s0:QROWS],
                        start=(j == 0),
                        stop=(j == KT - 1),
                    )
                rec = rcpool.tile([P, QROWS], F32, tag="rec")
                nc.vector.reciprocal_approx_fast(rec[:], pss[:, :QROWS])
                nc.vector.tensor_mul(xattnT[:, h, :], pso[:, :QROWS], rec[:])

        es_qkv.close()  # free kT/vN/qT

        # ---- phase E: o-proj + residual + rmsnorm2 ----
        es_xm = ExitStack()
        xmp = es_xm.enter_context(tc.tile_pool(name="xmp", bufs=1, side="right"))
        xmT = xmp.tile([P, DT, QROWS], BF16, tag="xmT")

        with nc.named_scope("E"), \
             tc.tile_pool(name="gpool2", bufs=3, side="left") as gpool2, \
             tc.tile_pool(name="spool2", bufs=4, side="left") as spool2, \
             tc.tile_pool(name="psbp2", bufs=2, space="PSUM") as psbp2:
            def _norm2_transpose(qt):
                mv = spool2.tile([P, 2], F32, tag="mv2", name="mv")
                nc.vector.bn_aggr(mv[:], stats2[:, qt])
                msq = spool2.tile([P, 1], F32, tag="msq2", name="msq")
                nc.vector.tensor_mul(msq[:], mv[:, 0:1], mv[:, 0:1])
                nc.vector.tensor_add(msq[:], msq[:], mv[:, 1:2])
                srt = spool2.tile([P, 1], F32, tag="srt2", name="srt")
                nc.scalar.activation(srt[:], msq[:], AF.Sqrt, bias=epsc[:])
                rstd = spool2.tile([P, 1], F32, tag="rstd2", name="rstd")
                nc.vector.reciprocal(rstd[:], srt[:])
                xn = gpool2.tile([P, c.D], BF16, tag="xn2", name="xn")
                nc.vector.tensor_scalar_mul(xn[:], res2[:, qt, :], rstd[:])
                for dt in range(DT):
                    tp = psbp2.tile([P, P], BF16, tag="psb2", name="psb")
                    nc.tensor.transpose(
                        tp[:], xn[:, dt * P : (dt + 1) * P], idb[:]
                    )
                    if dt % 2 == 0:
                        nc.scalar.copy(xmT[:, dt, qt * P : (qt + 1) * P], tp[:])
                    else:
                        nc.vector.tensor_copy(xmT[:, dt, qt * P : (qt + 1) * P], tp[:])

            stats2 = spool2.tile([P, QT, nsub, 6], F32, tag="stats2all")
            # last column group's weights stay resident so it can run
            # qt-outer: each qt's norm2 chain overlaps the next qt's matmuls
            LOG = len(OGS) - 1
            wotL = wstr2.tile([P, c.H, OGS[0][1]], BF16, tag="wotL", bufs=1)
            for ht in range(c.H):
                eng = nc.sync if ht % 2 == 0 else nc.scalar
                eng.dma_start(wotL[:, ht, :], wo_d[LOG, ht])
            for ogi, (os_, ow) in enumerate(OGS[:-1]):
                pss4 = [ps_tile() for _ in range(QT)]
                for ht in range(c.H):
                    if ogi == 0 and ht < NPRE_O:
                        wot = wot_pre[ht]
                    else:
                        wot = wstr2.tile([P, OGS[0][1]], BF16, tag="wot", name="wot")
                        eng = nc.sync if ht % 2 == 0 else nc.scalar
                        eng.dma_start(wot[:], wo_d[ogi, ht])
                    for qt in range(QT):
                        nc.tensor.matmul(
                            pss4[qt][:, :ow],
                            xattnT[:, ht, qt * P : (qt + 1) * P],
                            wot[:, :ow],
                            start=(ht == 0),
                            stop=(ht == c.H - 1),
                        )
                for qt in range(QT):
                    nc.vector.tensor_add(
                        res2[:, qt, os_ : os_ + ow],
                        pss4[qt][:, :ow],
                        xq_raw[:, qt, os_ : os_ + ow],
                    )
                    # incremental norm2 stats: OGS chunks == bn subgroups
                    nc.vector.bn_stats(
                        stats2[:, qt, ogi, :], res2[:, qt, os_ : os_ + ow]
                    )
            os_, ow = OGS[LOG]
            for qt in range(QT):
                psl = ps_tile()
                for ht in range(c.H):
                    nc.tensor.matmul(
                        psl[:, :ow],
                        xattnT[:, ht, qt * P : (qt + 1) * P],
                        wotL[:, ht, :ow],
                        start=(ht == 0),
                        stop=(ht == c.H - 1),
                    )
                nc.vector.tensor_add(
                    res2[:, qt, os_ : os_ + ow],
                    psl[:, :ow],
                    xq_raw[:, qt, os_ : os_ + ow],
                )
                nc.vector.bn_stats(
                    stats2[:, qt, LOG, :], res2[:, qt, os_ : os_ + ow]
                )
                _norm2_transpose(qt)

        es_wo.close()   # free wstr2 (top of left stack)
        es_attn.close()  # free xattnT

        # gating fold pieces: res2 := (res2 - xq_raw)*g + xq_raw, emitted
        # interleaved into phase F so they never block the PE
        fold_ops = []
        for qt in range(QT):
            fold_ops.append(lambda qt=qt: nc.vector.tensor_sub(
                res2[:, qt, :], res2[:, qt, :], xq_raw[:, qt, :]))
            fold_ops.append(lambda qt=qt: nc.vector.tensor_scalar_mul(
                res2[:, qt, :], res2[:, qt, :], gsc[:, qt : qt + 1]))
            fold_ops.append(lambda qt=qt: nc.vector.tensor_add(
                res2[:, qt, :], res2[:, qt, :], xq_raw[:, qt, :]))

        # ---- phase F: mlp gate/up ----
        es_act = ExitStack()
        actp = es_act.enter_context(tc.tile_pool(name="actp", bufs=1, side="left"))
        actT = actp.tile([P, c.FFT, QROWS], BF16, tag="actT")

        # down-proj weight pool created BEFORE F so its first DMAs prefetch
        # during the gate/up phase
        es_wd = ExitStack()
        wstr4 = es_wd.enter_context(tc.tile_pool(name="wstr4", bufs=8, side="left"))
        NPRE_D = 8
        wdt_pre = []
        for ffp in range(NPRE_D):
            wdt = wstr4.tile([P, OGS[0][1]], BF16, tag="wdt", name="wdt")
            nc.sync.dma_start(wdt[:], wd_d[0, ffp])
            wdt_pre.append(wdt)

        with nc.named_scope("F"), \
             tc.tile_pool(name="wstr3", bufs=8, side="left") as wstr3, \
             tc.tile_pool(name="psf", bufs=2, space="PSUM") as psf, \
             tc.tile_pool(name="fpool", bufs=3, side="left") as fpool:
            for gi in range(c.FFG):
                psg = [ps_tile() for _ in range(4)]
                for dt in range(DT):
                    wgt = wstr3.tile([P, 512], BF16, tag="wgut")
                    eng = nc.sync if dt % 2 == 0 else nc.scalar
                    eng.dma_start(wgt[:], wg_d[gi, dt])
                    for s in range(4):
                        nc.tensor.matmul(
                            psg[s][:, :QROWS],
                            wgt[:, s * P : (s + 1) * P],
                            xmT[:, dt, :],
                            start=(dt == 0),
                            stop=(dt == DT - 1),
                        )
                silu = fpool.tile([P, 4, QROWS], F32, tag="silu")
                for s in range(4):
                    # silu(x) = x * sigmoid(x)
                    nc.scalar.activation(silu[:, s, :], psg[s][:, :QROWS], AF.Sigmoid)
                    nc.vector.tensor_mul(silu[:, s, :], silu[:, s, :], psg[s][:, :QROWS])
                # up-proj accumulators: 2 from psp + 2 from psf so they never
                # WAR-wait on the gate accumulators' silu reads
                psu = [ps_tile(), ps_tile(),
                       psf.tile([P, 512], F32, tag="psf", name="ps"),
                       psf.tile([P, 512], F32, tag="psf", name="ps")]
                for dt in range(DT):
                    wut = wstr3.tile([P, 512], BF16, tag="wgut")
                    eng = nc.sync if dt % 2 == 0 else nc.scalar
                    eng.dma_start(wut[:], wu_d[gi, dt])
                    for s in range(4):
                        nc.tensor.matmul(
                            psu[s][:, :QROWS],
                            wut[:, s * P : (s + 1) * P],
                            xmT[:, dt, :],
                            start=(dt == 0),
                            stop=(dt == DT - 1),
                        )
                for s in range(4):
                    nc.vector.tensor_mul(
                        actT[:, gi * 4 + s, :], silu[:, s, :], psu[s][:, :QROWS]
                    )
                if gi < len(fold_ops):
                    fold_ops[gi]()

        es_xm.close()  # free xmT

        # ---- phase G: down-proj + residual + gating + output ----
        with nc.named_scope("G"), \
             tc.tile_pool(name="opool", bufs=3, side="left") as opool:
            for ogi, (os_, ow) in enumerate(OGS):
                psd = [ps_tile() for _ in range(QT)]
                for ffp in range(c.FFT):
                    if ogi == 0 and ffp < NPRE_D:
                        wdt = wdt_pre[ffp]
                    else:
                        wdt = wstr4.tile([P, OGS[0][1]], BF16, tag="wdt", name="wdt")
                        eng = nc.sync if ffp % 2 == 0 else nc.scalar
                        eng.dma_start(wdt[:], wd_d[ogi, ffp])
                    for qt in range(QT):
                        nc.tensor.matmul(
                            psd[qt][:, :ow],
                            actT[:, ffp, qt * P : (qt + 1) * P],
                            wdt[:, :ow],
                            start=(ffp == 0),
                            stop=(ffp == c.FFT - 1),
                        )
                for qt in range(QT):
                    t1 = opool.tile([P, 512], F32, tag="updt")
                    nc.vector.scalar_tensor_tensor(
                        t1[:, :ow],
                        psd[qt][:, :ow],
                        gsc[:, qt : qt + 1],
                        res2[:, qt, os_ : os_ + ow],
                        mybir.AluOpType.mult,
                        mybir.AluOpType.add,
                    )
                    # outputs alternate between the (otherwise idle) gpsimd
                    # dyn queue and the sync queue to halve the drain tail
                    oeng = nc.gpsimd if qt % 2 == 0 else nc.sync
                    oeng.dma_start(
                        oupd_d[qt * P : (qt + 1) * P, os_ : os_ + ow], t1[:, :ow]
                    )

        es_wd.close()
        es_act.close()
        es_res2.close()
    return nc


# ---------------- host side ----------------


def _bf(x):
    return np.ascontiguousarray(x.astype(BF16NP))


def _f32(x):
    return np.ascontiguousarray(x, dtype=np.float32)


def prep_shared(c: Cfg, Wq, bq, Wk, bk, Wv, bv, Wo, w_gate, w_up, w_down, ln1_w, ln2_w):
    """Host-side weight folding + tiling (exact fp32 math, then bf16 cast)."""
    DT, FFT, FFG, KVD = c.DT, c.FFT, c.FFG, c.KVH * c.HD
    OGS = _chunks(c.D, 512)
    OG, OW = len(OGS), OGS[0][1]
    Wqf = _f32(Wq) * _f32(ln1_w)[:, None]
    Wkf = _f32(Wk) * _f32(ln1_w)[:, None]
    Wvf = _f32(Wv) * _f32(ln1_w)[:, None]
    Wgf = _f32(w_gate) * _f32(ln2_w)[:, None]
    Wuf = _f32(w_up) * _f32(ln2_w)[:, None]

    perm = np.zeros((P, P), np.float32)
    half = c.HD // 2
    perm[np.arange(half) + half, np.arange(half)] = -1.0
    perm[np.arange(half), np.arange(half) + half] = 1.0

    # tri[k, q] = 1 iff k <= q (within-tile causal diagonal block)
    tri = np.triu(np.ones((P, P), np.float32))

    return dict(
        wq=_bf(Wqf.reshape(DT, P, c.H, c.HD).transpose(2, 1, 0, 3)),
        wk=_bf(Wkf.reshape(DT, P, c.KVH, c.HD).transpose(2, 1, 0, 3)),
        wv=_bf(Wvf.reshape(DT, P, KVD).transpose(1, 0, 2)),
        wo=_bf(_f32(Wo).reshape(c.H, P, OG, OW).transpose(2, 0, 1, 3)),
        wg=_bf(Wgf.reshape(DT, P, FFG, 512).transpose(2, 0, 1, 3)),
        wu=_bf(Wuf.reshape(DT, P, FFG, 512).transpose(2, 0, 1, 3)),
        wd=_bf(_f32(w_down).reshape(FFT, P, OG, OW).transpose(2, 0, 1, 3)),
        bq=np.ascontiguousarray(_f32(bq).reshape(c.H, P).T),
        bk=np.ascontiguousarray(_f32(bk).reshape(c.KVH, P).T),
        bv=_f32(bv).reshape(1, KVD),
        id_f=np.eye(P, dtype=np.float32),
        id_b=np.eye(P, dtype=np.float32).astype(BF16NP),
        perm=perm.astype(BF16NP),
        ones_b=np.ones((P, P), np.float32).astype(BF16NP),
        trim=tri.astype(BF16NP),
    )


def prep_core(c: Cfg, shared, hid_b, idx_b, g_b, cos_b, sin_b, h):
    """Per-core inputs for core handling query-half h of one batch."""
    QROWS, QT, KT = c.QROWS, c.QT, c.KT
    idx32 = idx_b.astype(np.int32)
    # permute keys so this core's own query half comes first; the other
    # half is then either fully-visible (h=1) or fully-masked (h=0)
    kperm = np.concatenate(
        [np.arange(h * QROWS, (h + 1) * QROWS),
         np.arange(0, h * QROWS), np.arange((h + 1) * QROWS, c.KSEL)]
    )
    idx32 = idx32[kperm]
    m = dict(
        hid=_f32(hid_b),
        idx_kv=np.ascontiguousarray(idx32.reshape(KT, P).T),
        gsc=np.ascontiguousarray(
            _f32(g_b[h * QROWS : (h + 1) * QROWS]).reshape(QT, P).T
        ),
        cosb=_f32(cos_b),
        sinb=_f32(sin_b),
        biasm=np.full((P, 1), 0.0 if h == 1 else -60.0, np.float32),
    )
    m.update(shared)
    return m


_NC_CACHE = {}


def _get_nc(c: Cfg):
    key = c
    if key not in _NC_CACHE:
        nc = bacc.Bacc()
        emit(nc, c)
        nc.compile()
        _NC_CACHE[key] = nc
    return _NC_CACHE[key]


_RUN_CACHE = {}


def _run_spmd_cached(c: Cfg, nc, in_maps):
    """run_bass_via_pjrt equivalent with a cached jitted executable.

    run_bass_kernel_spmd rebuilds its jit closure per call, so every kernel()
    invocation would re-trace + recompile (~40s).  Build the shard_map jit
    once per config and reuse it; repeat calls only pay host->device
    transfer + execution.
    """
    import jax
    import numpy as np
    from jax.sharding import Mesh, PartitionSpec
    from jax.experimental.shard_map import shard_map
    from concourse import bass2jax
    from concourse.bass2jax import _bass_exec_p, install_neuronx_cc_hook

    n_cores = len(in_maps)
    key = (c, n_cores)
    if key not in _RUN_CACHE:
        install_neuronx_cc_hook()
        partition_name = (
            nc.partition_id_tensor.name if nc.partition_id_tensor else None
        )
        in_names, out_names, out_avals = [], [], []
        for alloc in nc.m.functions[0].allocations:
            if not isinstance(alloc, mybir.MemoryLocationSet):
                continue
            name = alloc.memorylocations[0].name
            if alloc.kind == "ExternalInput":
                if name != partition_name:
                    in_names.append(name)
            elif alloc.kind == "ExternalOutput":
                out_names.append(name)
                out_avals.append(
                    jax.core.ShapedArray(
                        tuple(alloc.tensor_shape), mybir.dt.np(alloc.dtype)
                    )
                )
        n_params = len(in_names)
        all_in = list(in_names) + list(out_names)
        if partition_name is not None:
            all_in.append(partition_name)

        def _body(*flat):
            operands = list(flat)
            if partition_name is not None:
                operands.append(bass2jax.partition_id_tensor())
            return tuple(
                _bass_exec_p.bind(
                    *operands,
                    out_avals=tuple(out_avals),
                    in_names=tuple(all_in),
                    out_names=tuple(out_names),
                    lowering_input_output_aliases=(),
                    sim_require_finite=True,
                    sim_require_nnan=True,
                    nc=nc,
                )
            )

        devices = jax.devices()[:n_cores]
        mesh = Mesh(np.asarray(devices), ("core",))
        n_outs = len(out_avals)
        sharded = jax.jit(
            shard_map(
                _body,
                mesh=mesh,
                in_specs=(PartitionSpec("core"),) * (n_params + n_outs),
                out_specs=(PartitionSpec("core"),) * n_outs,
                check_rep=False,
            ),
            keep_unused=True,
        )
        zeros = [
            np.zeros((n_cores * a.shape[0], *a.shape[1:]), a.dtype)
            for a in out_avals
        ]
        _RUN_CACHE[key] = (sharded, in_names, out_names, out_avals, zeros)

    sharded, in_names, out_names, out_avals, zeros = _RUN_CACHE[key]
    concat_in = [
        np.concatenate([np.asarray(in_maps[ci][nm]) for ci in range(n_cores)], axis=0)
        for nm in in_names
    ]
    out_arrs = sharded(*concat_in, *zeros)
    return [
        {
            name: np.asarray(out_arrs[i]).reshape(n_cores, *out_avals[i].shape)[ci]
            for i, name in enumerate(out_names)
        }
        for ci in range(n_cores)
    ]


def kernel(
    hidden_states,
    topk_indices,
    gating_scores,
    cos,
    sin,
    Wq,
    bq,
    Wk,
    bk,
    Wv,
    bv,
    Wo,
    w_gate,
    w_up,
    w_down,
    ln1_w,
    ln2_w,
):
    c = FULL
    B = hidden_states.shape[0]
    hidden_states = np.asarray(hidden_states)
    topk_indices = np.asarray(topk_indices)
    shared = prep_shared(
        c, Wq, bq, Wk, bk, Wv, bv, Wo, w_gate, w_up, w_down, ln1_w, ln2_w
    )
    in_maps = []
    for b in range(B):
        for h in range(2):
            in_maps.append(
                prep_core(
                    c,
                    shared,
                    hidden_states[b],
                    topk_indices[b],
                    np.asarray(gating_scores)[b],
                    np.asarray(cos)[b],
                    np.asarray(sin)[b],
                    h,
                )
            )
    nc = _get_nc(c)
    res = _run_spmd_cached(c, nc, in_maps)

    # untouched rows pass through host-side; only updated rows come back
    final = np.array(hidden_states, dtype=np.float32, copy=True)
    for ci in range(len(in_maps)):
        b, h = ci // 2, ci % 2
        sel = topk_indices[b, h * c.QROWS : (h + 1) * c.QROWS].astype(np.int64)
        final[b, sel] = res[ci]["out_upd"]
    return final


# revision 32
# speedup vs baseline: 1.0073x; 1.0073x over previous
"""Trainium2 Bass kernel for the DynamicBlock (ragged top-k decoder layer).

Sharding: 8 cores = (batch b in 0..3) x (query-half h in 0..1).
Core (b, h) processes queries k in [h*512, (h+1)*512) of the K=1024 selected
rows of batch b (causal: needs K/V for all 1024 selected rows, computed
locally -- no collectives).  Matmuls run in bf16 with fp32 accumulation;
norms/softmax/residual/gating in fp32.

Key structure (v2):
- keys permuted own-half-first; the other half is either fully allowed
  (h=1) or fully masked (h=0), folded into the EXP bias per core; only the
  4 own diagonal blocks need an explicit triangular mask multiply.
- scores/exp/attn matmuls trimmed to the causal column range on own tiles.
- softmax 1/sum via reciprocal_approx_fast (single DVE op) straight out of
  PSUM; pso/pss live in a dedicated 2-bank PSUM pool so the next head's
  score matmuls never WAR-wait on the normalization chain.
- B/C interleaved for in-order engines: own gathers -> Q proj -> other
  gathers -> V -> K, so the PE starts ~40us earlier.
- weight pools for o-proj/down-proj created one phase early so their first
  DMAs prefetch across the phase boundary.
- untouched hidden rows are assembled host-side (no device copy-through).
"""

import math
from contextlib import ExitStack
from dataclasses import dataclass

import ml_dtypes
import numpy as np

import concourse.bass as bass
import concourse.mybir as mybir
import concourse.tile as tile
from concourse import bacc
from concourse.bass import IndirectOffsetOnAxis

P = 128
F32 = mybir.dt.float32
BF16 = mybir.dt.bfloat16
I32 = mybir.dt.int32
AF = mybir.ActivationFunctionType
BF16NP = ml_dtypes.bfloat16


@dataclass(frozen=True)
class Cfg:
    T: int = 4096      # full sequence length
    D: int = 2048      # model dim
    KSEL: int = 1024   # selected rows per sequence
    H: int = 16        # query heads
    KVH: int = 4       # kv heads
    HD: int = 128      # head dim (must equal P)
    FF: int = 8192     # mlp intermediate
    EPS: float = 1e-6

    @property
    def DT(self):
        return self.D // P

    @property
    def QROWS(self):
        return self.KSEL // 2

    @property
    def QT(self):
        return self.QROWS // P

    @property
    def KT(self):
        return self.KSEL // P

    @property
    def FFT(self):
        return self.FF // P

    @property
    def FFG(self):
        return self.FFT // 4

    @property
    def T2(self):
        return self.T // 2

    @property
    def GQ(self):
        return self.H // self.KVH


FULL = Cfg()


def _chunks(total, size):
    out = []
    s = 0
    while s < total:
        out.append((s, min(size, total - s)))
        s += size
    return out


def emit(nc: bass.Bass, c: Cfg, upto: str = "G"):
    DT, QT, KT, QROWS, KVD = c.DT, c.QT, c.KT, c.QROWS, c.KVH * c.HD
    OGS = _chunks(c.D, 512)  # output-column groups for o-proj / down-proj

    # ---- DRAM I/O ----
    hid_d = nc.dram_tensor("hid", [c.T, c.D], F32, kind="ExternalInput")
    idxkv_d = nc.dram_tensor("idx_kv", [P, KT], I32, kind="ExternalInput")
    gsc_d = nc.dram_tensor("gsc", [P, QT], F32, kind="ExternalInput")
    cos_d = nc.dram_tensor("cosb", [c.T, c.HD], F32, kind="ExternalInput")
    sin_d = nc.dram_tensor("sinb", [c.T, c.HD], F32, kind="ExternalInput")
    tri_d = nc.dram_tensor("trim", [P, P], BF16, kind="ExternalInput")
    biasm_d = nc.dram_tensor("biasm", [P, 1], F32, kind="ExternalInput")
    wq_d = nc.dram_tensor("wq", [c.H, P, DT, c.HD], BF16, kind="ExternalInput")
    wk_d = nc.dram_tensor("wk", [c.KVH, P, DT, c.HD], BF16, kind="ExternalInput")
    wv_d = nc.dram_tensor("wv", [P, DT, KVD], BF16, kind="ExternalInput")
    wo_d = nc.dram_tensor("wo", [len(OGS), c.H, P, OGS[0][1]], BF16, kind="ExternalInput")
    wg_d = nc.dram_tensor("wg", [c.FFG, DT, P, 512], BF16, kind="ExternalInput")
    wu_d = nc.dram_tensor("wu", [c.FFG, DT, P, 512], BF16, kind="ExternalInput")
    wd_d = nc.dram_tensor("wd", [len(OGS), c.FFT, P, OGS[0][1]], BF16, kind="ExternalInput")
    bq_d = nc.dram_tensor("bq", [P, c.H], F32, kind="ExternalInput")
    bk_d = nc.dram_tensor("bk", [P, c.KVH], F32, kind="ExternalInput")
    bv_d = nc.dram_tensor("bv", [1, KVD], F32, kind="ExternalInput")
    idf_d = nc.dram_tensor("id_f", [P, P], F32, kind="ExternalInput")
    idb_d = nc.dram_tensor("id_b", [P, P], BF16, kind="ExternalInput")
    perm_d = nc.dram_tensor("perm", [P, P], BF16, kind="ExternalInput")
    ones_d = nc.dram_tensor("ones_b", [P, P], BF16, kind="ExternalInput")

    oupd_d = nc.dram_tensor("out_upd", [QROWS, c.D], F32, kind="ExternalOutput")

    scl = 1.0 / math.sqrt(c.HD)

    with ExitStack() as top:
        tc = top.enter_context(tile.TileContext(nc))
        constp = top.enter_context(tc.tile_pool(name="constp", bufs=1, side="left"))
        residp = top.enter_context(tc.tile_pool(name="residp", bufs=1, side="left"))
        psp = top.enter_context(tc.tile_pool(name="psp", bufs=6, space="PSUM"))

        def ps_tile():
            return psp.tile([P, 512], F32, tag="ps", name="ps")

        # ---- constants (indices first: they gate the gathers) ----
        idxkv = constp.tile([P, KT], I32, tag="idxkv")
        nc.sync.dma_start(idxkv[:], idxkv_d[:])
        idf = constp.tile([P, P], F32, tag="idf")
        nc.sync.dma_start(idf[:], idf_d[:])
        idb = constp.tile([P, P], BF16, tag="idb")
        nc.sync.dma_start(idb[:], idb_d[:])
        perm = constp.tile([P, P], BF16, tag="perm")
        nc.sync.dma_start(perm[:], perm_d[:])
        ones_b = constp.tile([P, P], BF16, tag="ones_b")
        nc.sync.dma_start(ones_b[:], ones_d[:])
        tri = constp.tile([P, P], BF16, tag="tri")
        nc.sync.dma_start(tri[:], tri_d[:])
        biasm = constp.tile([P, 1], F32, tag="biasm")
        nc.sync.dma_start(biasm[:], biasm_d[:])
        gsc = constp.tile([P, QT], F32, tag="gsc")
        nc.sync.dma_start(gsc[:], gsc_d[:])
        bqc = constp.tile([P, c.H], F32, tag="bqc")
        nc.sync.dma_start(bqc[:], bq_d[:])
        bkc = constp.tile([P, c.KVH], F32, tag="bkc")
        nc.sync.dma_start(bkc[:], bk_d[:])
        epsc = constp.tile([P, 1], F32, tag="epsc")
        nc.vector.memset(epsc[:], c.EPS)
        bvbc = constp.tile([P, KVD], F32, tag="bvbc")
        bv_ap = bv_d[:]
        nc.sync.dma_start(
            bvbc[:], bass.AP(tensor=bv_ap.tensor, offset=0, ap=[[0, P], [1, KVD]])
        )

        # residual (live until the end)
        xq_raw = residp.tile([P, QT, c.D], F32, tag="xq_raw")

        sgw = math.gcd(512, c.D)
        nsub = c.D // sgw

        es_bt = ExitStack()  # xkvT/xqT/cos/sin: freed after projections
        xtp = es_bt.enter_context(tc.tile_pool(name="xtp", bufs=1, side="left"))
        xkvT = xtp.tile([P, DT, c.KSEL], BF16, tag="xkvT")
        cosTkv = xtp.tile([P, c.KSEL], F32, tag="cosTkv")
        sinTkv = xtp.tile([P, c.KSEL], F32, tag="sinTkv")
        cosg = xtp.tile([P, KT, c.HD], F32, tag="cosg")
        sing = xtp.tile([P, KT, c.HD], F32, tag="sing")
        # host permutes the key order so this core's own query half is rows
        # [0, QROWS) -- q-side tensors are static slices of the kv tensors
        xqT = xkvT[:, :, :QROWS]
        cosTq = cosTkv[:, :QROWS]
        sinTq = sinTkv[:, :QROWS]

        es_qkv = ExitStack()
        qkvp = es_qkv.enter_context(tc.tile_pool(name="qkvp", bufs=1, side="right"))
        kT = qkvp.tile([P, c.KVH, c.KSEL], BF16, tag="kT")
        vN = qkvp.tile([P, KT, KVD], BF16, tag="vN")
        qT = qkvp.tile([P, c.H, QROWS], BF16, tag="qT")

        def gather_rows(dst, src_dram, col, split=1):
            """Indirect row gather; dst slots are always fresh so the only
            dependency is the idx tile (single sync wait on the dyn queue).
            split>1 breaks the row into column chunks so more packets are in
            flight per DMA engine (hides per-descriptor latency)."""
            ncols = src_dram.shape[-1]
            step = ncols // split
            for s in range(split):
                nc.gpsimd.indirect_dma_start(
                    out=dst[:, s * step : (s + 1) * step],
                    out_offset=None,
                    in_=src_dram[:],
                    in_offset=IndirectOffsetOnAxis(ap=idxkv[:, col : col + 1], axis=0),
                    element_offset=s * step,
                )

        with tc.tile_pool(name="gpool", bufs=3, side="left") as gpool, \
             tc.tile_pool(name="spool", bufs=4, side="left") as spool, \
             tc.tile_pool(name="psbp", bufs=2, space="PSUM") as psbp, \
             tc.tile_pool(name="wstr", bufs=2, side="left") as wstr, \
             tc.tile_pool(name="rpool", bufs=2, side="left") as rpool:

            def psb_tile():
                return psbp.tile([P, P], BF16, tag="psb", name="psb")

            def norm_transpose(raw, xn_out_fn, deep_ps=False):
                """raw: [P, D] f32 tile; writes bf16 normalized transposed tiles.

                deep_ps routes transposes through the 6-deep psp pool (f32
                PSUM) instead of the 2-deep psbp, so the transpose rate is
                copy-throughput-paced rather than slot-turnaround-paced --
                used where the transposes are on the critical path."""
                stats = spool.tile([P, nsub, 6], F32, tag="stats")
                for s in range(nsub):
                    nc.vector.bn_stats(stats[:, s, :], raw[:, s * sgw : (s + 1) * sgw])
                mv = spool.tile([P, 2], F32, tag="mv")
                nc.vector.bn_aggr(mv[:], stats[:])
                msq = spool.tile([P, 1], F32, tag="msq")
                nc.vector.tensor_mul(msq[:], mv[:, 0:1], mv[:, 0:1])
                nc.vector.tensor_add(msq[:], msq[:], mv[:, 1:2])
                srt = spool.tile([P, 1], F32, tag="srt")
                nc.scalar.activation(srt[:], msq[:], AF.Sqrt, bias=epsc[:])
                rstd = spool.tile([P, 1], F32, tag="rstd")
                nc.vector.reciprocal(rstd[:], srt[:])
                xn = gpool.tile([P, c.D], BF16, tag="xn")
                nc.vector.tensor_scalar_mul(xn[:], raw[:], rstd[:])
                for dt in range(DT):
                    tp = psb_tile()[:]
                    nc.tensor.transpose(tp, xn[:, dt * P : (dt + 1) * P], idb[:])
                    # alternate copy engine so neither serializes the drain
                    if dt % 2 == 0:
                        nc.scalar.copy(xn_out_fn(dt), tp)
                    else:
                        nc.vector.tensor_copy(xn_out_fn(dt), tp)

            def cs_transpose(t):
                for ei, (src, dst) in enumerate(((cosg, cosTkv), (sing, sinTkv))):
                    tp = ps_tile()
                    nc.tensor.transpose(tp[:, :P], src[:, t, :], idf[:])
                    if ei == 0:
                        nc.scalar.copy(dst[:, t * P : (t + 1) * P], tp[:, :P])
                    else:
                        nc.vector.tensor_copy(dst[:, t * P : (t + 1) * P], tp[:, :P])

            def rope(dst, rawt, rot_ps, cosT, sinT, s0, w):
                t1 = rpool.tile([P, 512], F32, tag="ropet1")
                nc.vector.tensor_mul(t1[:, :w], rawt[:, s0 : s0 + w], cosT[:, s0 : s0 + w])
                t2 = rpool.tile([P, 512], F32, tag="ropet2")
                nc.vector.tensor_mul(t2[:, :w], rot_ps[:, :w], sinT[:, s0 : s0 + w])
                nc.vector.tensor_add(dst[:, s0 : s0 + w], t1[:, :w], t2[:, :w])

            # ---- phase B1: own-half gathers + rmsnorm1 + transpose ----
            with nc.named_scope("B1"):
                for t in range(QT):
                    raw = xq_raw[:, t, :]
                    gather_rows(raw, hid_d, t, split=2)
                    norm_transpose(
                        raw, lambda dt, t=t: xkvT[:, dt, t * P : (t + 1) * P]
                    )
                for t in range(QT):
                    gather_rows(cosg[:, t, :], cos_d, t)
                    gather_rows(sing[:, t, :], sin_d, t)

            # ---- phase C1: Q projection + rope ----
            with nc.named_scope("C1"):
                # the rot matmul for head m waits on head m's DVE bias-add;
                # defer it by one head so the in-order PE never stalls on it
                def q_rope(m, qraw):
                    rot = ps_tile()
                    nc.tensor.matmul(
                        rot[:, :QROWS], perm[:], qraw[:], start=True, stop=True
                    )
                    rope(qT[:, m, :], qraw, rot, cosTq, sinTq, 0, QROWS)

                prev_q = None
                for m in range(c.H):
                    wqm = wstr.tile([P, DT, c.HD], BF16, tag="wqkm", bufs=3)
                    nc.sync.dma_start(wqm[:], wq_d[m])
                    qraw = rpool.tile([P, QROWS], BF16, tag="kqraw", name="qraw")
                    ps = ps_tile()
                    for dt in range(DT):
                        nc.tensor.matmul(
                            ps[:, :QROWS],
                            wqm[:, dt, :],
                            xqT[:, dt, :],
                            start=(dt == 0),
                            stop=(dt == DT - 1),
                        )
                    nc.vector.tensor_scalar_add(
                        qraw[:], ps[:, :QROWS], bqc[:, m : m + 1]
                    )
                    if m == 0:
                        # own cos/sin transposes: data has landed by the time
                        # the PE finishes head 0's projection
                        for t in range(QT):
                            cs_transpose(t)
                    if prev_q is not None:
                        q_rope(*prev_q)
                    prev_q = (m, qraw)
                q_rope(*prev_q)

            # ---- phase B2 + C2a: other-half gathers interleaved with V ----
            # V-proj matmuls for the already-transposed B1 tiles run while
            # the other-half gathers land; each B2 tile's norm+transposes
            # then interleave with the next V-proj block so the PE never
            # sits in a copy-paced transpose run
            with nc.named_scope("B2"):
                wvsb = wstr.tile([P, DT, KVD], BF16, tag="wvsb", bufs=1)
                nc.sync.dma_start(wvsb[:], wv_d[:])
                graws = {}
                for t in range(QT, KT):
                    graws[t] = gpool.tile(
                        [P, c.D], F32, tag="graw", name="graw", bufs=3
                    )[:]
                    gather_rows(graws[t], hid_d, t, split=2)
                for t in range(QT, KT):
                    gather_rows(cosg[:, t, :], cos_d, t)
                    gather_rows(sing[:, t, :], sin_d, t)

                def v_proj(rt):
                    psv = ps_tile()
                    for dt in range(DT):
                        nc.tensor.matmul(
                            psv[:, :KVD],
                            xkvT[:, dt, rt * P : (rt + 1) * P],
                            wvsb[:, dt, :],
                            start=(dt == 0),
                            stop=(dt == DT - 1),
                        )
                    nc.vector.tensor_add(vN[:, rt, :], psv[:, :KVD], bvbc[:])

                for rt in range(QT):
                    v_proj(rt)
                for t in range(QT, KT):
                    norm_transpose(
                        graws[t], lambda dt, t=t: xkvT[:, dt, t * P : (t + 1) * P]
                    )
                    v_proj(t)

            # ---- phase C2: K projection + rope ----
            with nc.named_scope("C2"):
                for t in range(QT, KT):
                    cs_transpose(t)
                def k_rope(m, kraw):
                    for s0, w in _chunks(c.KSEL, 512):
                        rot = ps_tile()
                        nc.tensor.matmul(
                            rot[:, :w], perm[:], kraw[:, s0 : s0 + w], start=True, stop=True
                        )
                        rope(kT[:, m, :], kraw, rot, cosTkv, sinTkv, s0, w)

                prev_k = None
                for m in range(c.KVH):
                    wkm = wstr.tile([P, DT, c.HD], BF16, tag="wqkm", bufs=3)
                    nc.sync.dma_start(wkm[:], wk_d[m])
                    kraw = rpool.tile([P, c.KSEL], BF16, tag="kraw")
                    for s0, w in _chunks(c.KSEL, 512):
                        ps = ps_tile()
                        for dt in range(DT):
                            nc.tensor.matmul(
                                ps[:, :w],
                                wkm[:, dt, :],
                                xkvT[:, dt, s0 : s0 + w],
                                start=(dt == 0),
                                stop=(dt == DT - 1),
                            )
                        nc.vector.tensor_scalar_add(
                            kraw[:, s0 : s0 + w], ps[:, :w], bkc[:, m : m + 1]
                        )
                    if prev_k is not None:
                        k_rope(*prev_k)
                    prev_k = (m, kraw)
                k_rope(*prev_k)

        es_bt.close()  # free xkvT/xqT/cos/sin

        # ---- phase D: attention ----
        es_res2 = ExitStack()  # attn+mlp residual, lives D -> G
        res2p = es_res2.enter_context(tc.tile_pool(name="res2p", bufs=1, side="left"))
        res2 = res2p.tile([P, QT, c.D], F32, tag="res2")
        es_attn = ExitStack()
        attnp = es_attn.enter_context(tc.tile_pool(name="attnp", bufs=1, side="left"))
        xattnT = attnp.tile([P, c.H, QROWS], BF16, tag="xattnT")

        # o-proj weight pool created BEFORE D: its first DMAs prefetch
        # during the (DMA-idle) attention phase
        es_wo = ExitStack()
        wstr2 = es_wo.enter_context(tc.tile_pool(name="wstr2", bufs=16, side="left"))
        NPRE_O = 16
        wot_pre = []
        for ht in range(NPRE_O):
            wot = wstr2.tile([P, OGS[0][1]], BF16, tag="wot", name="wot")
            nc.sync.dma_start(wot[:], wo_d[0, ht])
            wot_pre.append(wot)

        with nc.named_scope("D"), \
             tc.tile_pool(name="dpool", bufs=3, side="left") as dpool, \
             tc.tile_pool(name="psov", bufs=2, space="PSUM") as psov, \
             tc.tile_pool(name="rcpool", bufs=2, side="left") as rcpool:
            for h in range(c.H):
                g = h // c.GQ
                expT = dpool.tile([P, KT, QROWS], BF16, tag="expT")
                # own-half tiles: causal-trimmed columns + diagonal tri mask
                for j in range(QT):
                    s0 = j * P
                    ps = ps_tile()
                    nc.tensor.matmul(
                        ps[:, s0:QROWS],
                        kT[:, g, s0 : s0 + P],
                        qT[:, h, s0:QROWS],
                        start=True,
                        stop=True,
                    )
                    nc.scalar.activation(
                        expT[:, j, s0:QROWS], ps[:, s0:QROWS], AF.Exp, scale=scl
                    )
                    nc.vector.tensor_mul(
                        expT[:, j, s0 : s0 + P], expT[:, j, s0 : s0 + P], tri[:]
                    )
                # other-half tiles: all-allowed (h=1) or all-masked (h=0),
                # folded into the exp bias (e^-60 ~ 0)
                for j in range(QT, KT):
                    ps = ps_tile()
                    nc.tensor.matmul(
                        ps[:, :QROWS],
                        kT[:, g, j * P : (j + 1) * P],
                        qT[:, h, :],
                        start=True,
                        stop=True,
                    )
                    nc.scalar.activation(
                        expT[:, j, :], ps[:, :QROWS], AF.Exp,
                        scale=scl, bias=biasm[:],
                    )
                pso = psov.tile([P, 512], F32, tag="pso", name="pso")
                pss = psov.tile([P, 512], F32, tag="pso", name="pss")
                for j in range(KT):
                    s0 = j * P if j < QT else 0
                    nc.tensor.matmul(
                        pso[:, s0:QROWS],
                        vN[:, j, g * c.HD : (g + 1) * c.HD],
                        expT[:, j, s0:QROWS],
                        start=(j == 0),
                        stop=(j == KT - 1),
                    )
                for j in range(KT):
                    s0 = j * P if j < QT else 0
                    nc.tensor.matmul(
                        pss[:, s0:QROWS],
                        ones_b[:],
                        expT[:, j, s0:QROWS],
                        start=(j == 0),
                        stop=(j == KT - 1),
                    )
                rec = rcpool.tile([P, QROWS], F32, tag="rec")
                nc.vector.reciprocal_approx_fast(rec[:], pss[:, :QROWS])
                nc.vector.tensor_mul(xattnT[:, h, :], pso[:, :QROWS], rec[:])

        es_qkv.close()  # free kT/vN/qT

        # ---- phase E: o-proj + residual + rmsnorm2 ----
        es_xm = ExitStack()
        xmp = es_xm.enter_context(tc.tile_pool(name="xmp", bufs=1, side="right"))
        xmT = xmp.tile([P, DT, QROWS], BF16, tag="xmT")

        with nc.named_scope("E"), \
             tc.tile_pool(name="gpool2", bufs=3, side="left") as gpool2, \
             tc.tile_pool(name="spool2", bufs=4, side="left") as spool2, \
             tc.tile_pool(name="psbp2", bufs=2, space="PSUM") as psbp2:
            def _norm2_transpose(qt):
                mv = spool2.tile([P, 2], F32, tag="mv2", name="mv")
                nc.vector.bn_aggr(mv[:], stats2[:, qt])
                msq = spool2.tile([P, 1], F32, tag="msq2", name="msq")
                nc.vector.tensor_mul(msq[:], mv[:, 0:1], mv[:, 0:1])
                nc.vector.tensor_add(msq[:], msq[:], mv[:, 1:2])
                srt = spool2.tile([P, 1], F32, tag="srt2", name="srt")
                nc.scalar.activation(srt[:], msq[:], AF.Sqrt, bias=epsc[:])
                rstd = spool2.tile([P, 1], F32, tag="rstd2", name="rstd")
                nc.vector.reciprocal(rstd[:], srt[:])
                xn = gpool2.tile([P, c.D], BF16, tag="xn2", name="xn")
                nc.vector.tensor_scalar_mul(xn[:], res2[:, qt, :], rstd[:])
                for dt in range(DT):
                    tp = psbp2.tile([P, P], BF16, tag="psb2", name="psb")
                    nc.tensor.transpose(
                        tp[:], xn[:, dt * P : (dt + 1) * P], idb[:]
                    )
                    if dt % 2 == 0:
                        nc.scalar.copy(xmT[:, dt, qt * P : (qt + 1) * P], tp[:])
                    else:
                        nc.vector.tensor_copy(xmT[:, dt, qt * P : (qt + 1) * P], tp[:])

            stats2 = spool2.tile([P, QT, nsub, 6], F32, tag="stats2all")
            # last column group's weights stay resident so it can run
            # qt-outer: each qt's norm2 chain overlaps the next qt's matmuls
            LOG = len(OGS) - 1
            wotL = wstr2.tile([P, c.H, OGS[0][1]], BF16, tag="wotL", bufs=1)
            for ht in range(c.H):
                eng = nc.sync if ht % 2 == 0 else nc.scalar
                eng.dma_start(wotL[:, ht, :], wo_d[LOG, ht])
            for ogi, (os_, ow) in enumerate(OGS[:-1]):
                pss4 = [ps_tile() for _ in range(QT)]
                for ht in range(c.H):
                    if ogi == 0 and ht < NPRE_O:
                        wot = wot_pre[ht]
                    else:
                        wot = wstr2.tile([P, OGS[0][1]], BF16, tag="wot", name="wot")
                        eng = nc.sync if ht % 2 == 0 else nc.scalar
                        eng.dma_start(wot[:], wo_d[ogi, ht])
                    for qt in range(QT):
                        nc.tensor.matmul(
                            pss4[qt][:, :ow],
                            xattnT[:, ht, qt * P : (qt + 1) * P],
                            wot[:, :ow],
                            start=(ht == 0),
                            stop=(ht == c.H - 1),
                        )
                for qt in range(QT):
                    nc.vector.tensor_add(
                        res2[:, qt, os_ : os_ + ow],
                        pss4[qt][:, :ow],
                        xq_raw[:, qt, os_ : os_ + ow],
                    )
                    # incremental norm2 stats: OGS chunks == bn subgroups
                    nc.vector.bn_stats(
                        stats2[:, qt, ogi, :], res2[:, qt, os_ : os_ + ow]
                    )
            os_, ow = OGS[LOG]
            for qt in range(QT):
                psl = ps_tile()
                for ht in range(c.H):
                    nc.tensor.matmul(
                        psl[:, :ow],
                        xattnT[:, ht, qt * P : (qt + 1) * P],
                        wotL[:, ht, :ow],
                        start=(ht == 0),
                        stop=(ht == c.H - 1),
                    )
                nc.vector.tensor_add(
                    res2[:, qt, os_ : os_ + ow],
                    psl[:, :ow],
                    xq_raw[:, qt, os_ : os_ + ow],
                )
                nc.vector.bn_stats(
                    stats2[:, qt, LOG, :], res2[:, qt, os_ : os_ + ow]
                )
                _norm2_transpose(qt)

        es_wo.close()   # free wstr2 (top of left stack)
        es_attn.close()  # free xattnT

        # gating fold pieces: res2 := (res2 - xq_raw)*g + xq_raw, emitted
        # interleaved into phase F so they never block the PE
        fold_ops = []
        for qt in range(QT):
            fold_ops.append(lambda qt=qt: nc.vector.tensor_sub(
                res2[:, qt, :], res2[:, qt, :], xq_raw[:, qt, :]))
            fold_ops.append(lambda qt=qt: nc.vector.tensor_scalar_mul(
                res2[:, qt, :], res2[:, qt, :], gsc[:, qt : qt + 1]))
            fold_ops.append(lambda qt=qt: nc.vector.tensor_add(
                res2[:, qt, :], res2[:, qt, :], xq_raw[:, qt, :]))

        # ---- phase F: mlp gate/up ----
        es_act = ExitStack()
        actp = es_act.enter_context(tc.tile_pool(name="actp", bufs=1, side="left"))
        actT = actp.tile([P, c.FFT, QROWS], BF16, tag="actT")

        # down-proj weight pool created BEFORE F so its first DMAs prefetch
        # during the gate/up phase
        es_wd = ExitStack()
        wstr4 = es_wd.enter_context(tc.tile_pool(name="wstr4", bufs=8, side="left"))
        NPRE_D = 8
        wdt_pre = []
        for ffp in range(NPRE_D):
            wdt = wstr4.tile([P, OGS[0][1]], BF16, tag="wdt", name="wdt")
            nc.sync.dma_start(wdt[:], wd_d[0, ffp])
            wdt_pre.append(wdt)

        with nc.named_scope("F"), \
             tc.tile_pool(name="wstr3", bufs=8, side="left") as wstr3, \
             tc.tile_pool(name="psf", bufs=2, space="PSUM") as psf, \
             tc.tile_pool(name="fpool", bufs=3, side="left") as fpool:
            for gi in range(c.FFG):
                psg = [ps_tile() for _ in range(4)]
                for dt in range(DT):
                    wgt = wstr3.tile([P, 512], BF16, tag="wgut")
                    eng = nc.sync if dt % 2 == 0 else nc.scalar
                    eng.dma_start(wgt[:], wg_d[gi, dt])
                    for s in range(4):
                        nc.tensor.matmul(
                            psg[s][:, :QROWS],
                            wgt[:, s * P : (s + 1) * P],
                            xmT[:, dt, :],
                            start=(dt == 0),
                            stop=(dt == DT - 1),
                        )
                silu = fpool.tile([P, 4, QROWS], F32, tag="silu")
                for s in range(4):
                    # silu(x) = x * sigmoid(x)
                    nc.scalar.activation(silu[:, s, :], psg[s][:, :QROWS], AF.Sigmoid)
                    nc.vector.tensor_mul(silu[:, s, :], silu[:, s, :], psg[s][:, :QROWS])
                # up-proj accumulators: 2 from psp + 2 from psf so they never
                # WAR-wait on the gate accumulators' silu reads
                psu = [ps_tile(), ps_tile(),
                       psf.tile([P, 512], F32, tag="psf", name="ps"),
                       psf.tile([P, 512], F32, tag="psf", name="ps")]
                for dt in range(DT):
                    wut = wstr3.tile([P, 512], BF16, tag="wgut")
                    eng = nc.sync if dt % 2 == 0 else nc.scalar
                    eng.dma_start(wut[:], wu_d[gi, dt])
                    for s in range(4):
                        nc.tensor.matmul(
                            psu[s][:, :QROWS],
                            wut[:, s * P : (s + 1) * P],
                            xmT[:, dt, :],
                            start=(dt == 0),
                            stop=(dt == DT - 1),
                        )
                for s in range(4):
                    nc.vector.tensor_mul(
                        actT[:, gi * 4 + s, :], silu[:, s, :], psu[s][:, :QROWS]
                    )
                if gi < len(fold_ops):
                    fold_ops[gi]()

        es_xm.close()  # free xmT

        # ---- phase G: down-proj + residual + gating + output ----
        with nc.named_scope("G"), \
             tc.tile_pool(name="opool", bufs=3, side="left") as opool:
            for ogi, (os_, ow) in enumerate(OGS):
                psd = [ps_tile() for _ in range(QT)]
                for ffp in range(c.FFT):
                    if ogi == 0 and ffp < NPRE_D:
                        wdt = wdt_pre[ffp]
                    else:
                        wdt = wstr4.tile([P, OGS[0][1]], BF16, tag="wdt", name="wdt")
                        eng = nc.sync if ffp % 2 == 0 else nc.scalar
                        eng.dma_start(wdt[:], wd_d[ogi, ffp])
                    for qt in range(QT):
                        nc.tensor.matmul(
                            psd[qt][:, :ow],
                            actT[:, ffp, qt * P : (qt + 1) * P],
                            wdt[:, :ow],
                            start=(ffp == 0),
                            stop=(ffp == c.FFT - 1),
                        )
                for qt in range(QT):
                    t1 = opool.tile([P, 512], F32, tag="updt")
                    nc.vector.scalar_tensor_tensor(
                        t1[:, :ow],
                        psd[qt][:, :ow],
                        gsc[:, qt : qt + 1],
                        res2[:, qt, os_ : os_ + ow],
                        mybir.AluOpType.mult,
                        mybir.AluOpType.add,
                    )
                    # outputs alternate between the (otherwise idle) gpsimd
                    # dyn queue and the sync queue to halve the drain tail
                    oeng = nc.gpsimd if qt % 2 == 0 else nc.sync
                    oeng.dma_start(
                        oupd_d[qt * P : (qt + 1) * P, os_ : os_ + ow], t1[:, :ow]
                    )

        es_wd.close()
        es_act.close()
        es_res2.close()
    return nc


# ---------------- host side ----------------


def _bf(x):
    return np.ascontiguousarray(x.astype(BF16NP))


def _f32(x):
    return np.ascontiguousarray(x, dtype=np.float32)


def prep_shared(c: Cfg, Wq, bq, Wk, bk, Wv, bv, Wo, w_gate, w_up, w_down, ln1_w, ln2_w):
    """Host-side weight folding + tiling (exact fp32 math, then bf16 cast)."""
    DT, FFT, FFG, KVD = c.DT, c.FFT, c.FFG, c.KVH * c.HD
    OGS = _chunks(c.D, 512)
    OG, OW = len(OGS), OGS[0][1]
    Wqf = _f32(Wq) * _f32(ln1_w)[:, None]
    Wkf = _f32(Wk) * _f32(ln1_w)[:, None]
    Wvf = _f32(Wv) * _f32(ln1_w)[:, None]
    Wgf = _f32(w_gate) * _f32(ln2_w)[:, None]
    Wuf = _f32(w_up) * _f32(ln2_w)[:, None]

    perm = np.zeros((P, P), np.float32)
    half = c.HD // 2
    perm[np.arange(half) + half, np.arange(half)] = -1.0
    perm[np.arange(half), np.arange(half) + half] = 1.0

    # tri[k, q] = 1 iff k <= q (within-tile causal diagonal block)
    tri = np.triu(np.ones((P, P), np.float32))

    return dict(
        wq=_bf(Wqf.reshape(DT, P, c.H, c.HD).transpose(2, 1, 0, 3)),
        wk=_bf(Wkf.reshape(DT, P, c.KVH, c.HD).transpose(2, 1, 0, 3)),
        wv=_bf(Wvf.reshape(DT, P, KVD).transpose(1, 0, 2)),
        wo=_bf(_f32(Wo).reshape(c.H, P, OG, OW).transpose(2, 0, 1, 3)),
        wg=_bf(Wgf.reshape(DT, P, FFG, 512).transpose(2, 0, 1, 3)),
        wu=_bf(Wuf.reshape(DT, P, FFG, 512).transpose(2, 0, 1, 3)),
        wd=_bf(_f32(w_down).reshape(FFT, P, OG, OW).transpose(2, 0, 1, 3)),
        bq=np.ascontiguousarray(_f32(bq).reshape(c.H, P).T),
        bk=np.ascontiguousarray(_f32(bk).reshape(c.KVH, P).T),
        bv=_f32(bv).reshape(1, KVD),
        id_f=np.eye(P, dtype=np.float32),
        id_b=np.eye(P, dtype=np.float32).astype(BF16NP),
        perm=perm.astype(BF16NP),
        ones_b=np.ones((P, P), np.float32).astype(BF16NP),
        trim=tri.astype(BF16NP),
    )


def prep_core(c: Cfg, shared, hid_b, idx_b, g_b, cos_b, sin_b, h):
    """Per-core inputs for core handling query-half h of one batch."""
    QROWS, QT, KT = c.QROWS, c.QT, c.KT
    idx32 = idx_b.astype(np.int32)
    # permute keys so this core's own query half comes first; the other
    # half is then either fully-visible (h=1) or fully-masked (h=0)
    kperm = np.concatenate(
        [np.arange(h * QROWS, (h + 1) * QROWS),
         np.arange(0, h * QROWS), np.arange((h + 1) * QROWS, c.KSEL)]
    )
    idx32 = idx32[kperm]
    m = dict(
        hid=_f32(hid_b),
        idx_kv=np.ascontiguousarray(idx32.reshape(KT, P).T),
        gsc=np.ascontiguousarray(
            _f32(g_b[h * QROWS : (h + 1) * QROWS]).reshape(QT, P).T
        ),
        cosb=_f32(cos_b),
        sinb=_f32(sin_b),
        biasm=np.full((P, 1), 0.0 if h == 1 else -60.0, np.float32),
    )
    m.update(shared)
    return m


_NC_CACHE = {}


def _get_nc(c: Cfg):
    key = c
    if key not in _NC_CACHE:
        nc = bacc.Bacc()
        emit(nc, c)
        nc.compile()
        _NC_CACHE[key] = nc
    return _NC_CACHE[key]


_RUN_CACHE = {}


def _run_spmd_cached(c: Cfg, nc, in_maps):
    """run_bass_via_pjrt equivalent with a cached jitted executable.

    run_bass_kernel_spmd rebuilds its jit closure per call, so every kernel()
    invocation would re-trace + recompile (~40s).  Build the shard_map jit
    once per config and reuse it; repeat calls only pay host->device
    transfer + execution.
    """
    import jax
    import numpy as np
    from jax.sharding import Mesh, PartitionSpec
    from jax.experimental.shard_map import shard_map
    from concourse import bass2jax
    from concourse.bass2jax import _bass_exec_p, install_neuronx_cc_hook

    n_cores = len(in_maps)
    key = (c, n_cores)
    if key not in _RUN_CACHE:
        install_neuronx_cc_hook()
        partition_name = (
            nc.partition_id_tensor.name if nc.partition_id_tensor else None
        )
        in_names, out_names, out_avals = [], [], []
        for alloc in nc.m.functions[0].allocations:
            if not isinstance(alloc, mybir.MemoryLocationSet):
                continue
            name = alloc.memorylocations[0].name
            if alloc.kind == "ExternalInput":
                if name != partition_name:
                    in_names.append(name)
            elif alloc.kind == "ExternalOutput":
                out_names.append(name)
                out_avals.append(
                    jax.core.ShapedArray(
                        tuple(alloc.tensor_shape), mybir.dt.np(alloc.dtype)
                    )
                )
        n_params = len(in_names)
        all_in = list(in_names) + list(out_names)
        if partition_name is not None:
            all_in.append(partition_name)

        def _body(*flat):
            operands = list(flat)
            if partition_name is not None:
                operands.append(bass2jax.partition_id_tensor())
            return tuple(
                _bass_exec_p.bind(
                    *operands,
                    out_avals=tuple(out_avals),
                    in_names=tuple(all_in),
                    out_names=tuple(out_names),
                    lowering_input_output_aliases=(),
                    sim_require_finite=True,
                    sim_require_nnan=True,
                    nc=nc,
                )
            )

        devices = jax.devices()[:n_cores]
        mesh = Mesh(np.asarray(devices), ("core",))
        n_outs = len(out_avals)
        sharded = jax.jit(
            shard_map(
                _body,
                mesh=mesh,
                in_specs=(PartitionSpec("core"),) * (n_params + n_outs),
                out_specs=(PartitionSpec("core"),) * n_outs,
                check_rep=False,
            ),
            keep_unused=True,
        )
        zeros = [
            np.zeros((n_cores * a.shape[0], *a.shape[1:]), a.dtype)
            for a in out_avals
        ]
        _RUN_CACHE[key] = (sharded, in_names, out_names, out_avals, zeros)

    sharded, in_names, out_names, out_avals, zeros = _RUN_CACHE[key]
    concat_in = [
        np.concatenate([np.asarray(in_maps[ci][nm]) for ci in range(n_cores)], axis=0)
        for nm in in_names
    ]
    out_arrs = sharded(*concat_in, *zeros)
    return [
        {
            name: np.asarray(out_arrs[i]).reshape(n_cores, *out_avals[i].shape)[ci]
            for i, name in enumerate(out_names)
        }
        for ci in range(n_cores)
    ]


def kernel(
    hidden_states,
    topk_indices,
    gating_scores,
    cos,
    sin,
    Wq,
    bq,
    Wk,
    bk,
    Wv,
    bv,
    Wo,
    w_gate,
    w_up,
    w_down,
    ln1_w,
    ln2_w,
):
    c = FULL
    B = hidden_states.shape[0]
    hidden_states = np.asarray(hidden_states)
    topk_indices = np.asarray(topk_indices)
    shared = prep_shared(
        c, Wq, bq, Wk, bk, Wv, bv, Wo, w_gate, w_up, w_down, ln1_w, ln2_w
    )
    in_maps = []
    for b in range(B):
        for h in range(2):
            in_maps.append(
                prep_core(
                    c,
                    shared,
                    hidden_states[b],
                    topk_indices[b],
                    np.asarray(gating_scores)[b],
                    np.asarray(cos)[b],
                    np.asarray(sin)[b],
                    h,
                )
            )
    nc = _get_nc(c)
    res = _run_spmd_cached(c, nc, in_maps)

    # untouched rows pass through host-side; only updated rows come back
    final = np.array(hidden_states, dtype=np.float32, copy=True)
    for ci in range(len(in_maps)):
        b, h = ci // 2, ci % 2
        sel = topk_indices[b, h * c.QROWS : (h + 1) * c.QROWS].astype(np.int64)
        final[b, sel] = res[ci]["out_upd"]
    return final
